# revision 61
# baseline (speedup 1.0000x reference)
"""Trainium2 Bass kernel for conv-attention (B=8, N=3136, C=192, 4 heads).

Sharding: data-parallel over batch, 1 batch element per NeuronCore (8 cores).
Per core: q depthwise conv on DVE as a tensor_scalar(4x-mode) leaf +
Pool-engine add tree, kv depthwise conv on the PE via diagonal-weight
matmuls, folded pointwise+BN+projection matmuls on PE (bf16), softmax exp
split 2:1 between ACT (exact, straight from PSUM) and DVE (Schraudolph
int16 bit-trick, valid since scores are in [-1, 1]), PV transposed to
[n, d] with an augmented ones-column for the denominator, reciprocal +
bulk broadcast-multiply normalization on DVE, output DMA'd directly as
[N, C]. Only ACT/DVE touch PSUM (hardware rule); Pool handles SBUF-only
work; projections are interleaved mid-window to keep the in-order PE
queue from stalling the exp pipeline.
"""

import numpy as np
import ml_dtypes

import concourse.bass as bass
import concourse.bacc as bacc
import concourse.mybir as mybir
import concourse.tile as tile
from concourse.bass_utils import run_bass_kernel_spmd

BF16 = ml_dtypes.bfloat16

C = 192
H = W = 56
N = H * W            # 3136
MO = 28
M = MO * MO          # 784
NH = 4
HD = C // NH         # 48
EPS = 1e-5
PAD = 58             # padded image row stride
PADN = PAD * PAD     # 3364
XBASE = 64           # image offset inside the SBUF x buffer
XW = XBASE + PADN + XBASE  # 3492
NW = 4               # n windows
WIN = 784            # window n size (14 image rows)
WROWS = 14
MT = 112             # m tile
NMT = 7              # m tiles
VW = NH * (HD + 1)   # 196, v with interleaved ones columns
PH = 841             # phase plane block size (29*29)
PHW = 4 * PH         # 3364

# Schraudolph exp constants for bf16 bit-domain (scores are in [-1, 1])
EXP_A = float(2.0 ** 7 / np.log(2.0))
EXP_B = float(127 * 2.0 ** 7 - 4.8)

_CACHE = {}


def _build_bass():
    fp32 = mybir.dt.float32
    bf16 = mybir.dt.bfloat16
    i16 = mybir.dt.int16
    nc = bacc.Bacc(None)

    # ---- external I/O (per core) ----
    xe = nc.dram_tensor("xe", [128, XW], bf16, kind="ExternalInput")
    xe2 = nc.dram_tensor("xe2", [64, XW], bf16, kind="ExternalInput")
    phAd = nc.dram_tensor("phA", [128, PHW], bf16, kind="ExternalInput")
    phBd = nc.dram_tensor("phB", [64, PHW], bf16, kind="ExternalInput")
    dwkAd = nc.dram_tensor("dwkA", [128, 9 * 128], bf16, kind="ExternalInput")
    dwkBd = nc.dram_tensor("dwkB", [64, 9 * 64], bf16, kind="ExternalInput")
    dwqAd = nc.dram_tensor("dwqA", [128, 9 * 128], bf16, kind="ExternalInput")
    dwqBd = nc.dram_tensor("dwqB", [64, 9 * 64], bf16, kind="ExternalInput")
    lqA = nc.dram_tensor("lqA", [128, 256], bf16, kind="ExternalInput")
    lqB = nc.dram_tensor("lqB", [65, 256], bf16, kind="ExternalInput")
    lkA = nc.dram_tensor("lkA", [128, 256], bf16, kind="ExternalInput")
    lkB = nc.dram_tensor("lkB", [65, 256], bf16, kind="ExternalInput")
    rvA = nc.dram_tensor("rvA", [128, VW], bf16, kind="ExternalInput")
    rvB = nc.dram_tensor("rvB", [65, VW], bf16, kind="ExternalInput")
    wq = nc.dram_tensor("wq", [192, 9], fp32, kind="ExternalInput")
    od = nc.dram_tensor("o", [N, C], fp32, kind="ExternalOutput")

    AF = mybir.ActivationFunctionType
    AL = mybir.AluOpType

    with tile.TileContext(nc) as tc:
        with (
            tc.tile_pool(name="xbuf", bufs=1) as xbuf,
            tc.tile_pool(name="wt", bufs=1) as wt,
            tc.tile_pool(name="z", bufs=1) as zp,
            tc.tile_pool(name="tmp", bufs=3) as tmp,
            tc.tile_pool(name="qk", bufs=1) as qk,
            tc.tile_pool(name="vs", bufs=1) as vsp,
            tc.tile_pool(name="slabp", bufs=3, space="PSUM") as slabp,
            tc.tile_pool(name="pvp", bufs=1, space="PSUM") as pvp,
            tc.tile_pool(name="pp", bufs=8) as pp,
            tc.tile_pool(name="dn", bufs=4) as dnp,
            tc.tile_pool(name="op", bufs=2) as outp,
        ):
            # ---- load inputs (order matters: kv-diag weights + phases
            # first on sync so the PE prologue starts ASAP; x + q weights
            # first on gpsimd for the DVE conv) ----
            phA1 = xbuf.tile([128, 2 * PH], bf16, tag="phA1")
            phA2 = xbuf.tile([128, 2 * PH], bf16, tag="phA2")
            phB1 = xbuf.tile([64, 2 * PH], bf16, tag="phB1")
            phB2 = xbuf.tile([64, 2 * PH], bf16, tag="phB2")
            dwkA = wt.tile([128, 9 * 128], bf16, tag="dwkA")
            dwkB = wt.tile([64, 9 * 64], bf16, tag="dwkB")
            nc.sync.dma_start(phA1[:], phAd[:, 0:2 * PH])
            nc.sync.dma_start(dwkA[:], dwkAd[:])
            nc.sync.dma_start(dwkB[:], dwkBd[:])
            nc.sync.dma_start(phA2[:], phAd[:, 2 * PH:PHW])
            nc.sync.dma_start(phB1[:], phBd[:, 0:2 * PH])
            nc.sync.dma_start(phB2[:], phBd[:, 2 * PH:PHW])
            lk_A = wt.tile([128, 256], bf16, tag="lkA")
            lk_B = wt.tile([65, 256], bf16, tag="lkB")
            rv_A = wt.tile([128, VW], bf16, tag="rvA")
            rv_B = wt.tile([65, VW], bf16, tag="rvB")
            for t, d in ((lk_A, lkA), (lk_B, lkB), (rv_A, rvA), (rv_B, rvB)):
                nc.sync.dma_start(t[:], d[:])
            xeB = xbuf.tile([64, XW], bf16, tag="xeB")

            xeA = xbuf.tile([128, XW], bf16, tag="xeA")
            wqt = wt.tile([128, 9], fp32, tag="wq")
            wqt2 = wt.tile([64, 9], fp32, tag="wq2")
            nc.gpsimd.dma_start(wqt[:], wq[0:128, :])
            nc.gpsimd.dma_start(wqt2[:], wq[128:192, :])
            nc.gpsimd.dma_start(xeA[:], xe[:])
            lq_A = wt.tile([128, 256], bf16, tag="lqA")
            lq_B = wt.tile([65, 256], bf16, tag="lqB")
            nc.scalar.dma_start(xeB[:], xe2[:])
            nc.scalar.dma_start(lq_A[:], lqA[:])
            nc.scalar.dma_start(lq_B[:], lqB[:])
            dwqA = wt.tile([128, 9 * 128], bf16, tag="dwqA")
            dwqB = wt.tile([64, 9 * 64], bf16, tag="dwqB")
            nc.sync.dma_start(dwqA[:], dwqAd[:])
            nc.sync.dma_start(dwqB[:], dwqBd[:])

            # preload the Exp activation table during the lead-in
            warm = tmp.tile([1, 1], fp32, tag="warm")
            nc.vector.memset(warm[:], 0.0)
            nc.scalar.activation(warm[:], warm[:], AF.Exp)

            # ---- conv helpers: tensor_scalar (4x) leaves + tensor_tensor
            # (2x) adds. Temps are fixed-shape [128, 812] with shared tags;
            # pool bufs=2 gives cross-window pipelining.
            def conv_tree(eng, srcs, wts, out_ap, prow, cols, leaf_view=None):
                # srcs: list of 9 source APs; wts: [prow, 9] weights
                ts = []
                for k2 in range(9):
                    t = tmp.tile([128, 812], bf16, tag=f"cvt{k2}")
                    tv = t[0:prow, 0:cols]
                    eng.tensor_scalar(
                        leaf_view(tv) if leaf_view else tv, srcs[k2],
                        wts[0:prow, k2:k2 + 1], None, AL.mult)
                    ts.append(t)
                # pairwise add tree: 9 -> 5 -> 3 -> 2 -> 1; the first-level
                # adds run on the otherwise idle Pool engine (SBUF-only)
                lvl = 0
                while len(ts) > 1:
                    nxt = []
                    for i2 in range(0, len(ts) - 1, 2):
                        aeng = nc.gpsimd
                        if len(ts) == 2:
                            nc.gpsimd.tensor_tensor(
                                out_ap, ts[i2][0:prow, 0:cols],
                                ts[i2 + 1][0:prow, 0:cols], AL.add)
                        else:
                            d = tmp.tile([128, 812], bf16, tag=f"cvu{lvl}_{i2}")
                            aeng.tensor_tensor(
                                d[0:prow, 0:cols], ts[i2][0:prow, 0:cols],
                                ts[i2 + 1][0:prow, 0:cols], AL.add)
                            nxt.append(d)
                    if len(ts) % 2:
                        nxt.append(ts[-1])
                    ts = nxt
                    lvl += 1

            # ---- q depthwise conv per window (variable row ranges) ----
            zqA = zp.tile([128, PADN], bf16, tag="zqA")
            zqB = zp.tile([65, PADN], bf16, tag="zqB")
            nc.gpsimd.memset(zqB[64:65, :], 1.0)

            # windows: (image row start, rows). Window 0 is split so the
            # first exp isn't gated on a full 14-row DVE conv.
            WINS = [(0, 8), (8, 6), (14, 14), (28, 14), (42, 14)]

            def dwq_window(win):
                r0w, nrw = win
                lo = 58 * r0w + 58
                width = 58 * nrw
                for prow, zt, xeT, wts in (
                        (128, zqA, xeA, wqt),
                        (64, zqB, xeB, wqt2)):
                    srcs = []
                    for di in range(3):
                        for dj in range(3):
                            d = (di - 1) * PAD + (dj - 1)
                            off = XBASE + d + lo
                            srcs.append(xeT[0:prow, off:off + width])
                    conv_tree(nc.vector, srcs, wts,
                              zt[0:prow, lo:lo + width], prow, width)

            def zq_chunk(zt, prow, r0, nr):
                base = 58 * (r0 + 1) + 1
                return zt[0:prow, base:base + 58 * nr].rearrange(
                    "p (a b) -> p a b", a=nr)[:, :, 0:56]

            dwq_window(WINS[0])

            # ---- kv depthwise conv on PE via diagonal weight matmuls ----
            zkA = zp.tile([128, M], bf16, tag="zkA")
            zkB = zp.tile([65, M], bf16, tag="zkB")
            nc.gpsimd.memset(zkB[64:65, :], 1.0)

            def kv_src(ph12, prow, di, dj):
                pr, roff = (1, 0) if di == 0 else (0, 0) if di == 1 else (1, 1)
                pc, coff = (1, 0) if dj == 0 else (0, 0) if dj == 1 else (1, 1)
                b = 2 * pr + pc
                pht = ph12[b // 2]
                base = PH * (b % 2)
                return pht[0:prow, base:base + PH].rearrange(
                    "p (a b) -> p a b", a=29)[:, roff:roff + 28, coff:coff + 28]

            TAP_ORDER = [(1, 0), (1, 1), (1, 2), (0, 0), (0, 1), (0, 2),
                         (2, 0), (2, 1), (2, 2)]
            for pht, prow, dwk, zk in (((phA1, phA2), 128, dwkA, zkA),
                                       ((phB1, phB2), 64, dwkB, zkB)):
                pz = slabp.tile([128, 1024], fp32, tag="slab", name="pz")
                for ti, (di, dj) in enumerate(TAP_ORDER):
                    t = 3 * di + dj
                    src = kv_src(pht, prow, di, dj)
                    for pi, (c0, r0, nr) in enumerate(((0, 0, 16), (512, 16, 12))):
                        nc.tensor.matmul(
                            pz[0:prow, c0:c0 + 28 * nr],
                            dwk[:, prow * t:prow * t + prow],
                            src[:, r0:r0 + nr, :],
                            start=(ti == 0), stop=(ti == 8))
                nc.scalar.copy(zk[0:prow, 0:448], pz[0:prow, 0:448])
                nc.scalar.copy(zk[0:prow, 448:M], pz[0:prow, 512:848])

            # ---- k projection: kT[112, 784] x2; mg1 (second head pair)
            # isn't consumed until mid-window-0, so it's deferred out of
            # the first-exp critical chain ----
            kTa = qk.tile([112, M], bf16, tag="kTa")
            kTb = qk.tile([112, M], bf16, tag="kTb")

            def emit_kproj(mg, kt):
                pk = slabp.tile([128, 1024], fp32, tag="slab", name="pk")
                for (c0, z0, cw) in ((0, 0, 512), (512, 512, 272)):
                    nc.tensor.matmul(pk[0:112, c0:c0 + cw],
                                     lk_A[:, 128 * mg:128 * mg + 112],
                                     zkA[:, z0:z0 + cw], start=True, stop=False)
                    nc.tensor.matmul(pk[0:112, c0:c0 + cw],
                                     lk_B[:, 128 * mg:128 * mg + 112],
                                     zkB[:, z0:z0 + cw], start=False, stop=True)
                nc.scalar.copy(kt[:, 0:512], pk[0:112, 0:512])
                nc.scalar.copy(kt[:, 512:M], pk[0:112, 512:784])

            emit_kproj(0, kTa)

            # ---- v projection: interleaved v_aug [112, 196] x 7; only
            # mt0 is needed before the first PV, the rest are deferred ----
            vS = vsp.tile([MT, NMT * VW], bf16, tag="vS")

            def emit_vproj(mt):
                pk = slabp.tile([128, 1024], fp32, tag="slab", name="pk")
                nc.tensor.matmul(pk[0:112, 0:VW], zkA[:, MT * mt:MT * mt + MT],
                                 rv_A[:], start=True, stop=False)
                nc.tensor.matmul(pk[0:112, 0:VW], zkB[:, MT * mt:MT * mt + MT],
                                 rv_B[:], start=False, stop=True)
                nc.scalar.copy(vS[:, VW * mt:VW * mt + VW], pk[0:112, 0:VW])

            emit_vproj(0)

            # ---- main pipeline ----
            qTa = qk.tile([112, N], bf16, tag="qTa")
            qTb = qk.tile([112, N], bf16, tag="qTb")
            pending = []  # deferred divide/dma emitters
            uidx = [0]  # exp unit counter: 2 of every 7 units go to DVE

            def emit_qproj_mg(win, mg, qt):
                r0w, nrw = win
                n0 = 56 * r0w
                if nrw <= 8:
                    pieces = [(0, r0w, nrw)]
                else:
                    pieces = [(0, r0w, 8), (512, r0w + 8, nrw - 8)]
                pq = slabp.tile([128, 1024], fp32, tag="slab", name="pq")
                for (c0, r0, nr) in pieces:
                    nc.tensor.matmul(pq[0:112, c0:c0 + 56 * nr],
                                     lq_A[:, 128 * mg:128 * mg + 112],
                                     zq_chunk(zqA, 128, r0, nr),
                                     start=True, stop=False)
                    nc.tensor.matmul(pq[0:112, c0:c0 + 56 * nr],
                                     lq_B[:, 128 * mg:128 * mg + 112],
                                     zq_chunk(zqB, 65, r0, nr),
                                     start=False, stop=True)
                off = 0
                ceng = nc.scalar.copy if r0w <= 8 else nc.vector.tensor_copy
                for (c0, r0, nr) in pieces:
                    ceng(qt[:, n0 + off:n0 + off + 56 * nr],
                         pq[0:112, c0:c0 + 56 * nr])
                    off += 56 * nr

            def emit_qproj(win):
                emit_qproj_mg(win, 0, qTa)
                emit_qproj_mg(win, 1, qTb)

            emit_qproj_mg(WINS[0], 0, qTa)
            delayed_pv = [None]  # 1-unit-delayed PV emitter
            for wi, win in enumerate(WINS):
                r0w, nrw = win
                n0 = 56 * r0w
                nlen = 56 * nrw
                nsub = nlen // MT
                if wi + 1 < len(WINS):
                    dwq_window(WINS[wi + 1])
                otw = outp.tile([112, NMT * C], fp32, tag="otw", name="otw")
                if nlen <= 512:
                    qk_pieces = [(0, nlen)]
                else:
                    qk_pieces = [(0, 512), (512, nlen - 512)]
                for hp, (kt, qt) in enumerate(((kTa, qTa), (kTb, qTb))):
                    pv = pvp.tile([128, 1024], fp32, tag="pv", name="pv")
                    for mt in range(NMT):
                        for h2 in range(2):
                            r0 = 64 * h2
                            slab = slabp.tile([128, 1024], fp32, tag="slab",
                                              name="slab")
                            for (c0, cw) in qk_pieces:
                                nc.tensor.matmul(
                                    slab[0:112, c0:c0 + cw],
                                    kt[r0:r0 + 48, MT * mt:MT * mt + MT],
                                    qt[r0:r0 + 48, n0 + c0:n0 + c0 + cw],
                                    start=True, stop=True)
                            pt = pp.tile([112, WIN], i16, tag=f"pt{h2}")
                            if uidx[0] % 3 == 2:
                                # Pool can't read PSUM; DVE takes the
                                # Schraudolph share
                                nc.vector.tensor_scalar(
                                    pt[:, 0:nlen], slab[0:112, 0:nlen],
                                    EXP_A, EXP_B, AL.mult, AL.add)
                            else:
                                nc.scalar.activation(
                                    pt[:, 0:nlen].bitcast(bf16),
                                    slab[0:112, 0:nlen], AF.Exp)
                            uidx[0] += 1
                            h = 2 * hp + h2
                            pb = pt[:].bitcast(bf16)

                            # PV batch, delayed by one unit so the next QK
                            # issues on the PE before this exp-dependent
                            # batch stalls the in-order PE queue. One psum
                            # accumulation group per bank: start marks the
                            # whole 2KB zero region, so only the first
                            # matmul in the bank starts and only the last
                            # stops.
                            def mk_pv(pv, pb, mt, h2, h, nsub):
                                def emit():
                                    for k in range(nsub):
                                        nc.tensor.matmul(
                                            pv[0:112, 512 * h2 + 49 * k:
                                               512 * h2 + 49 * k + 49],
                                            pb[:, MT * k:MT * k + MT],
                                            vS[:, VW * mt + 49 * h:
                                               VW * mt + 49 * h + 49],
                                            start=(mt == 0 and k == 0),
                                            stop=(mt == NMT - 1 and
                                                  k == nsub - 1))
                                return emit

                            if delayed_pv[0] is not None:
                                delayed_pv[0]()
                            # divides queued for a pv bank whose closing PV
                            # was just emitted by the delayed call above
                            for fn in pending:
                                fn()
                            del pending[:]
                            delayed_pv[0] = mk_pv(pv, pb, mt, h2, h, nsub)
                            # deferred prologue work (second head-pair
                            # projections + remaining v tiles), spread over
                            # window 0's early units
                            if wi == 0:
                                if (hp, mt, h2) == (0, 0, 1):
                                    emit_vproj(1)
                                    emit_vproj(2)
                                    emit_vproj(3)
                                elif (hp, mt, h2) == (0, 1, 0):
                                    emit_kproj(1, kTb)
                                elif (hp, mt, h2) == (0, 1, 1):
                                    emit_qproj_mg(WINS[0], 1, qTb)
                                elif (hp, mt, h2) == (0, 2, 1):
                                    emit_vproj(4)
                                    emit_vproj(5)
                                    emit_vproj(6)
                            # emit next window's q-proj mid-window so it
                            # doesn't bubble the PE stream at the boundary
                            if (hp, mt, h2) == (1, 2, 1) and wi + 1 < len(WINS):
                                emit_qproj(WINS[wi + 1])

                            # queue this head's bulk divide once its pv
                            # bank's last PV (emitted one unit later) closes
                            last_win = (wi == len(WINS) - 1)

                            def mk_div(pv, otw, n0, nsub, hp, h2,
                                       last_win=last_win):
                                def emit():
                                    h = 2 * hp + h2
                                    cb = 512 * h2
                                    grp = pv[0:112, cb:cb + 49 * nsub].rearrange(
                                        "p (k c) -> p k c", c=49)
                                    # no divide ALU on the DVE: gather the
                                    # denominators to SBUF, reciprocal, then
                                    # bulk broadcast multiplies. The final
                                    # (hp1, h2=1) group of the last window is
                                    # split by k so the output DMA overlaps
                                    # the remaining divides.
                                    dt = dnp.tile([112, 8], fp32,
                                                  tag=f"dn{h2}", name="dt")
                                    nc.vector.tensor_copy(
                                        dt[:, 0:nsub], grp[:, :, HD])
                                    rc = dnp.tile([112, 8], fp32,
                                                  tag=f"rc{h2}", name="rc")
                                    nc.vector.reciprocal_approx_fast(
                                        rc[:, 0:nsub], dt[:, 0:nsub])
                                    final = hp == 1 and h2 == 1
                                    chunks = ([(0, 4), (4, nsub)]
                                              if final and last_win and nsub > 4
                                              else [(0, nsub)])
                                    for (k0, k1) in chunks:
                                        nk = k1 - k0
                                        num = grp[:, k0:k1, 0:HD]
                                        den = rc[:, k0:k1].unsqueeze(
                                            2).broadcast_to([112, nk, HD])
                                        out = otw[:, k0 * C:k1 * C].rearrange(
                                            "p (k c) -> p k c", c=C)[
                                                :, :, HD * h:HD * h + HD]
                                        nc.vector.tensor_tensor(
                                            out, num, den, AL.mult)
                                        if final:
                                            dst = od[n0 + MT * k0:
                                                     n0 + MT * k1, :].rearrange(
                                                "(k p) c -> p k c", p=MT)
                                            eng = nc.sync if k0 == 0 else nc.gpsimd
                                            eng.dma_start(
                                                dst,
                                                otw[:, k0 * C:k1 * C].rearrange(
                                                    "p (k c) -> p k c", c=C))
                                return emit

                            if mt == NMT - 1:
                                pending.append(
                                    mk_div(pv, otw, n0, nsub, hp, h2))
            if delayed_pv[0] is not None:
                delayed_pv[0]()
            for fn in pending:
                fn()

    nc.finalize()
    return nc


def _host_prep(x, H_, W_, dw_q, g_q, b_q, m_q, v_q, pw_q,
               dw_kv, g_kv, b_kv, m_kv, v_kv, pw_kv,
               Wq, bq, Wk, bk, Wv, bv):
    f64 = np.float64
    s_q = (g_q / np.sqrt(v_q + EPS)).astype(f64)
    t_q = b_q.astype(f64) - m_q.astype(f64) * s_q
    s_k = (g_kv / np.sqrt(v_kv + EPS)).astype(f64)
    t_k = b_kv.astype(f64) - m_kv.astype(f64) * s_k
    pq2 = pw_q[:, :, 0, 0].astype(f64)
    pkv2 = pw_kv[:, :, 0, 0].astype(f64)
    scale = HD ** -0.5

    Bq = (Wq.astype(f64) @ pq2) * s_q[None, :] * scale
    cq = (Wq.astype(f64) @ (pq2 @ t_q) + bq.astype(f64)) * scale
    Bk = (Wk.astype(f64) @ pkv2[:C]) * s_k[None, :]
    ck = Wk.astype(f64) @ (pkv2[:C] @ t_k) + bk.astype(f64)
    Bv = (Wv.astype(f64) @ pkv2[C:]) * s_k[None, :]
    cv = Wv.astype(f64) @ (pkv2[C:] @ t_k) + bv.astype(f64)

    def pad_lhsT(Bm, cvec):
        full = np.vstack([Bm.T, cvec[None, :]])  # [193, 192]
        padded = np.zeros((193, 256), np.float64)
        for mg in range(2):
            padded[:, 128 * mg + 0:128 * mg + 48] = full[:, 96 * mg + 0:96 * mg + 48]
            padded[:, 128 * mg + 64:128 * mg + 112] = full[:, 96 * mg + 48:96 * mg + 96]
        return padded[0:128].astype(BF16), padded[128:193].astype(BF16)

    lqA, lqB = pad_lhsT(Bq, cq)
    lkA, lkB = pad_lhsT(Bk, ck)

    rv = np.zeros((C, VW), f64)
    rb = np.zeros((1, VW), f64)
    for h in range(NH):
        rv[:, 49 * h:49 * h + 48] = Bv.T[:, 48 * h:48 * h + 48]
        rb[0, 49 * h:49 * h + 48] = cv[48 * h:48 * h + 48]
        rb[0, 49 * h + 48] = 1.0
    rvA = rv[0:128].astype(BF16)
    rvB = np.vstack([rv[128:192], rb]).astype(BF16)

    wqc = dw_q[:, 0].reshape(C, 9).astype(np.float32)
    wkc = dw_kv[:, 0].reshape(C, 9).astype(np.float32)

    # diagonal conv weights for the PE: per tap, diag(w[:, t]) per group
    def diag_w(wc):
        dA = np.zeros((128, 9 * 128), np.float32)
        dB = np.zeros((64, 9 * 64), np.float32)
        for t in range(9):
            dA[np.arange(128), 128 * t + np.arange(128)] = wc[0:128, t]
            dB[np.arange(64), 64 * t + np.arange(64)] = wc[128:192, t]
        return dA.astype(BF16), dB.astype(BF16)

    dwkA, dwkB = diag_w(wkc)
    dwqA, dwqB = diag_w(wqc)

    B = x.shape[0]
    xpads = []
    for b in range(B):
        xb = np.ascontiguousarray(x[b].T).reshape(C, H, W)
        xp = np.zeros((C, PAD, PAD), np.float32)
        xp[:, 1:-1, 1:-1] = xb
        flat = np.zeros((C, XW), np.float32)
        flat[:, XBASE:XBASE + PADN] = xp.reshape(C, PADN)
        xe = flat.astype(BF16)
        # phase planes for the stride-2 kv conv: 4 blocks of [29, 29];
        # block (pr, pc): pr=0 -> input rows 2r, pr=1 -> rows 2r-1
        ph = np.zeros((C, 4, 29, 29), np.float32)
        xpad2 = np.zeros((C, H + 2, W + 2), np.float32)
        xpad2[:, 1:-1, 1:-1] = xb
        for pr in range(2):
            rows = (np.arange(29) * 2) if pr == 0 else (np.arange(29) * 2 - 1)
            rmask = rows <= H
            for pc in range(2):
                cols = (np.arange(29) * 2) if pc == 0 else (np.arange(29) * 2 - 1)
                cmask = cols <= W
                rr = rows[rmask]
                cc = cols[cmask]
                blk = xpad2[:, rr + 1][:, :, cc + 1]
                ph[:, 2 * pr + pc, :len(rr), :len(cc)] = blk
        phf = ph.reshape(C, PHW).astype(BF16)
        xpads.append((xe, phf))
    return (lqA, lqB, lkA, lkB, rvA, rvB, wqc, dwkA, dwkB, dwqA, dwqB, xpads)


_OUT_NAMES = ("o",)


def _core_in_map(prep, b):
    lqA, lqB, lkA, lkB, rvA, rvB, wqc, dwkA, dwkB, dwqA, dwqB, xpads = prep
    xe, phf = xpads[b]
    return {
        "xe": np.ascontiguousarray(xe[0:128]),
        "xe2": np.ascontiguousarray(xe[128:192]),
        "phA": np.ascontiguousarray(phf[0:128]),
        "phB": np.ascontiguousarray(phf[128:192]),
        "lqA": lqA, "lqB": lqB, "lkA": lkA, "lkB": lkB,
        "rvA": rvA, "rvB": rvB, "wq": wqc,
        "dwkA": dwkA, "dwkB": dwkB, "dwqA": dwqA, "dwqB": dwqB,
    }


def _assemble_output(res):
    B = len(res)
    out = np.empty((B, N, C), np.float32)
    for b in range(B):
        out[b] = res[b]["o"]
    return out


def _run(inputs, trace=False, tmpdir=None):
    x = np.asarray(inputs["x"], np.float32)
    B = x.shape[0]
    prep = _host_prep(
        x, inputs["H"], inputs["W"], *[np.asarray(inputs[k], np.float32) for k in (
            "dw_q", "bn_q_gamma", "bn_q_beta", "bn_q_mean", "bn_q_var", "pw_q",
            "dw_kv", "bn_kv_gamma", "bn_kv_beta", "bn_kv_mean", "bn_kv_var",
            "pw_kv", "Wq", "bq", "Wk", "bk", "Wv", "bv")])

    if "nc" not in _CACHE:
        _CACHE["nc"] = _build_bass()
    nc = _CACHE["nc"]

    in_maps = [_core_in_map(prep, b) for b in range(B)]
    bkr = run_bass_kernel_spmd(nc, in_maps, list(range(B)),
                               trace=trace, tmpdir=tmpdir)
    return _assemble_output(bkr.results), bkr


def kernel(**inputs):
    return _run(inputs)[0]

